# revision 38
# baseline (speedup 1.0000x reference)
"""Trainium2 Bass kernel for a class-weighted focal loss (CLASSNetLoss).

Reference math (per element, p = clip(x, 1e-5, 0.99999), w_c = c+1):
    pos = -(SS - w) * log(p) * (1-p)^2      if t > 0
    neg = -w       * log(1-p) * p^2         if t == 0
    out = 10 * mean(where(t>0, pos, neg) / SS),  SS = 210

Key reformulation (t in {0,1}):
    r   = t ? p : (1-p) = |x + t - 1|        (one select => ONE log)
    E   = clip(log r, >= log 1e-5) * (1-r)^2 (shared by both branches)
    raw = -(SS*t + w_c*(1-2t)) * E
    sum(raw) = -SS*sum(t*E) - sum_c w_c * colsum((1-2t)*E)_c

and with sgn = 2t-1 (exact in bf16):
    v = x + (t-1),  r = v * sgn = |v|   (abs as one 2x bf16 multiply)
    M = E * sgn = -(1-2t)*E

So each core only needs two per-class column sums, colsum(E) and colsum(M);
the host applies class weights and the -SS term (cs_tE = (cs_E + cs_M)/2).

Per-core layout: batch shard [65536, 20] viewed as [128 partitions x 10240],
where free index f has class (f mod 20).  Column sums run on the otherwise
idle TensorE with a ones[128,1] stationary operand, accumulating N=320
column blocks (320 % 20 == 0, <= one PSUM bank) into PSUM [1,320]
accumulators, so PSUM column j holds class (j mod 20).

Shipped variant "v7" (chunk=1280, bufs=(6,6,4,3), aq=0.75, skew=1):
  DMA   HWDGE raw f32/i32 loads (measured concurrent floor ~27.0us =
        10.49 MB/core at ~389 GB/s per-NC HBM; HWDGE avoids SWDGE's Q7
        descriptor path, which couples with DVE 2-port modes and
        degrades more under neighbor load)
  w = t - x via ONE f32-rate stt (merge + cast to fp16), so that
        |w| = 1-r feeds Ln directly, w^2 = (1-r)^2 needs NO abs, the
        p-clip folds into the Ln bias (l0 = ln(1 + 1e-5 - |w|), bias
        passed as a [P,1] AP), and g2 = (w>=0) = t masks the second
        accumulator m2 = e*g2 = t*E (combine mode "te").
  DVE   w stt (1x f32), g2 ts (4x), 25% of a=|w| (max(-w,w) stt),
        e=l0*s, m2=e*g2                            (~27.8us nominal)
  ACT   75% of Abs, Ln, Square (one `natural_log` table set, load
        hoisted out of the loop)                   (~27.9us nominal)
  PE    column-sum matmuls into 2 PSUM accumulators   hidden
Front stages (DMA/w/g2/a/s/l0) are emitted `skew` chunks AHEAD of back
stages (e/m2/matmul) — without the skew the per-chunk DVE->ACT->DVE
dependency chain makes the two engines alternate in lockstep (v3
measured ~40.7us; a pure 3-ACT-pass ablation alone measured 34.7us).
Runner-up variant "v6" (SWDGE cast-to-fp16 loads, no f32 op, nominal
engines ~22us but cast-DMA floor ~28.9us): won quiet-window batches
(33.7us med / 29.1us p25 vs v3's 39.3-40.7us) but lost the final two
loaded-window head-to-heads to v7 (39.3/38.8 vs 43.0/40.6 p25).
Absolute slope numbers drift 33-55us with neighbor load on the
tunneled device — compare configs only within one cross-config
interleaved bench batch (bench.py).
"""

from contextlib import ExitStack

import numpy as np

import concourse.bacc as bacc
import concourse.tile as tile
from concourse import mybir
from concourse.bass_utils import run_bass_kernel_spmd

B, C = 524288, 20
NCORES = 8
BS = B // NCORES            # 65536 batch rows per core
P = 128                     # SBUF partitions
FD = BS * C // P            # 10240 free elements per partition
CHUNK = 2560                # free elems per pipeline chunk (multiple of 20)
NCH = FD // CHUNK           # 4 chunks
NMM = 320                   # matmul free size (multiple of 20, <=512)
SS = 210.0
LOG_LO = float(np.log(np.float32(1e-5)))    # -11.512925
W = np.arange(1, C + 1, dtype=np.float64)   # class weights

F32 = mybir.dt.float32
BF16 = mybir.dt.bfloat16
F16 = mybir.dt.float16
I32 = mybir.dt.int32
Alu = mybir.AluOpType
Act = mybir.ActivationFunctionType

# host-side scale applied to the second accumulator to recover
# cs_ME = colsum((1-2t) * E) for each variant
M2_SCALE = {"gp_u": 2.0, "v2": -1.0, "v3": 2.0, "v4": -1.0,
            "v6": 1.0, "v7": 1.0, "v8": 1.0, "eng": 1.0}
DEFAULT_VARIANT = "v7"


def build_bass(
    variant: str = DEFAULT_VARIANT,
    reps: int = 1,
    loop_n: int = 0,
    stages: int = 4,
    clamp_on: str = "gp",
    cast_dma: bool = False,
    bufs: tuple = (6, 6, 5, 4),
    chunk: int = 1280,
    gp_cols: int = 0,
    staggered: bool = False,
    aq: float = 0.75,
    sq: float = 1.0,
    n_act: int = 3,
    n_dve: int = 4,
    in_dt: str = "f16",
    dma_chunk: int = 0,
    skew: int = 2,
    res_on: str = "dve",
) -> bacc.Bacc:
    """Per-core SPMD program.

    variant "v2": bf16 pipeline, abs via sign-multiply, GPSIMD does the
    1-input casts; "gp_u": f32 r, GPSIMD does the 2-input add (slower,
    kept as a proven-correct fallback).
    `reps` statically unrolls the body; `loop_n` > 0 wraps it in a dynamic
    For_i loop (timing amplification only).  `stages` < 4 ablates stages
    for engine attribution.
    """
    nc = bacc.Bacc(None, debug=False)
    x = nc.dram_tensor("output", [BS, C], F32, kind="ExternalInput")
    t = nc.dram_tensor("target", [BS, C], I32, kind="ExternalInput")
    out = nc.dram_tensor("partials", [1, 2 * NMM], F32, kind="ExternalOutput")

    xv = x[:].rearrange("(p f) c -> p (f c)", p=P)   # [128, 10240] f32
    tv = t[:].rearrange("(p f) c -> p (f c)", p=P)   # [128, 10240] i32

    b_in, b_cast, b_mid, b_out = bufs

    with ExitStack() as ctx:
        tc = ctx.enter_context(tile.TileContext(nc))
        singles = ctx.enter_context(tc.tile_pool(name="singles", bufs=1))
        apool = ctx.enter_context(tc.tile_pool(name="a", bufs=b_in))
        tpool = ctx.enter_context(tc.tile_pool(name="t", bufs=b_in))
        upool = ctx.enter_context(tc.tile_pool(name="u", bufs=b_cast))
        bpool = ctx.enter_context(tc.tile_pool(name="b", bufs=b_mid))
        opool = ctx.enter_context(tc.tile_pool(name="o", bufs=b_out))
        psum = ctx.enter_context(tc.tile_pool(name="ps", bufs=2, space="PSUM"))

        ones_dt = (F16 if in_dt == "f16" else BF16) \
            if variant in ("v6", "v7", "v8") else BF16
        ones = singles.tile([P, 1], ones_dt)
        nc.vector.memset(ones, 1.0)
        # bias AP for the clamped log: ln(1 + 1e-5 - |w|)
        lnb = singles.tile([P, 1], F32)
        nc.vector.memset(lnb, 1.0 + 1e-5)
        # bias APs for v8: Abs(u - 1) and Ln(a + 1e-5)
        m1b = singles.tile([P, 1], F32)
        nc.vector.memset(m1b, -1.0)
        epsb = singles.tile([P, 1], F32)
        nc.vector.memset(epsb, 1e-5)

        def chunk_v2(ci, nch, ps_e, ps_m):
            sl = slice(ci * chunk, (ci + 1) * chunk)
            if cast_dma:
                # SWDGE casts during the DMA itself; no engine pass needed.
                xb = apool.tile([P, chunk], BF16, tag="xb")
                tb = tpool.tile([P, chunk], BF16, tag="tb")
                nc.gpsimd.dma_start(out=xb, in_=xv[:, sl])
                nc.gpsimd.dma_start(out=tb, in_=tv[:, sl])
            else:
                xt = apool.tile([P, chunk], F32, tag="xt")
                ti = tpool.tile([P, chunk], I32, tag="ti")
                nc.sync.dma_start(out=xt, in_=xv[:, sl])
                nc.sync.dma_start(out=ti, in_=tv[:, sl])
            if stages < 1:
                return
            if not cast_dma:
                xb = upool.tile([P, chunk], BF16, tag="xb")
                nc.gpsimd.tensor_copy(out=xb, in_=xt)
            # tm = t - 1 in {-1, 0};  sgn = 2t - 1 in {-1, +1}
            tm = upool.tile([P, chunk], BF16, tag="tm")
            sg = upool.tile([P, chunk], BF16, tag="sg")
            tsrc = tb if cast_dma else ti
            nc.gpsimd.tensor_scalar(
                out=tm, in0=tsrc, scalar1=-1.0, scalar2=None,
                op0=Alu.add, op1=Alu.bypass,
            )
            nc.gpsimd.tensor_scalar(
                out=sg, in0=tsrc, scalar1=2.0, scalar2=-1.0,
                op0=Alu.mult, op1=Alu.add,
            )
            # v = x + (t-1);  r = v * sgn = |x + t - 1|
            v = upool.tile([P, chunk], BF16, tag="v")
            nc.vector.tensor_add(v, xb, tm)
            r = upool.tile([P, chunk], BF16, tag="r")
            nc.vector.tensor_mul(r, v, sg)
            if stages < 2:
                return
            l0 = bpool.tile([P, chunk], BF16, tag="l0")
            nc.scalar.activation(l0, r, Act.Ln)
            s = bpool.tile([P, chunk], BF16, tag="s")
            nc.scalar.activation(s, r, Act.Square, bias=1.0, scale=-1.0)
            if stages < 3:
                return
            # lc = max(l0, log 1e-5)  (clamps the r==0 -> -inf case)
            lc = bpool.tile([P, chunk], BF16, tag="lc")
            if clamp_on == "gp":
                nc.gpsimd.tensor_scalar(
                    out=lc, in0=l0, scalar1=LOG_LO, scalar2=None,
                    op0=Alu.max, op1=Alu.bypass,
                )
            else:
                nc.vector.tensor_scalar(
                    out=lc, in0=l0, scalar1=LOG_LO, scalar2=None,
                    op0=Alu.max, op1=Alu.bypass,
                )
            e = bpool.tile([P, chunk], BF16, tag="e")
            nc.vector.tensor_mul(e, lc, s)
            m2 = bpool.tile([P, chunk], BF16, tag="m2")
            nc.vector.tensor_mul(m2, e, sg)
            if stages < 4:
                return
            for j in range(chunk // NMM):
                js = slice(j * NMM, (j + 1) * NMM)
                first = ci == 0 and j == 0
                last = ci == nch - 1 and j == chunk // NMM - 1
                nc.tensor.matmul(
                    ps_e[0:1, :], ones, e[:, js], start=first, stop=last
                )
                nc.tensor.matmul(
                    ps_m[0:1, :], ones, m2[:, js], start=first, stop=last
                )

        def chunk_gp_u(ci, nch, ps_e, ps_m):
            sl = slice(ci * chunk, (ci + 1) * chunk)
            xt = apool.tile([P, chunk], F32, tag="xt")
            ti = tpool.tile([P, chunk], I32, tag="ti")
            nc.sync.dma_start(out=xt, in_=xv[:, sl])
            nc.sync.dma_start(out=ti, in_=tv[:, sl])
            if stages < 1:
                return
            u = upool.tile([P, chunk], F32, tag="u")
            nc.gpsimd.tensor_add(u, xt, ti)
            if stages < 2:
                return
            r = upool.tile([P, chunk], F32, tag="r")
            nc.scalar.activation(r, u, Act.Abs, bias=1.0, scale=-1.0)
            l0 = bpool.tile([P, chunk], BF16, tag="l0")
            nc.scalar.activation(l0, r, Act.Ln)
            s = bpool.tile([P, chunk], BF16, tag="s")
            nc.scalar.activation(s, r, Act.Square, bias=1.0, scale=-1.0)
            if stages < 3:
                return
            g = bpool.tile([P, chunk], BF16, tag="g")
            nc.vector.tensor_scalar(
                out=g, in0=ti, scalar1=-1.0, scalar2=0.5,
                op0=Alu.mult, op1=Alu.add,
            )
            e = bpool.tile([P, chunk], BF16, tag="e")
            nc.vector.scalar_tensor_tensor(
                out=e, in0=l0, scalar=LOG_LO, in1=s,
                op0=Alu.max, op1=Alu.mult,
            )
            m2 = bpool.tile([P, chunk], BF16, tag="m2")
            nc.vector.tensor_mul(m2, e, g)
            if stages < 4:
                return
            for j in range(chunk // NMM):
                js = slice(j * NMM, (j + 1) * NMM)
                first = ci == 0 and j == 0
                last = ci == nch - 1 and j == chunk // NMM - 1
                nc.tensor.matmul(
                    ps_e[0:1, :], ones, e[:, js], start=first, stop=last
                )
                nc.tensor.matmul(
                    ps_m[0:1, :], ones, m2[:, js], start=first, stop=last
                )

        def chunk_v3(ci, nch, ps_e, ps_m):
            sl = slice(ci * chunk, (ci + 1) * chunk)
            xt = apool.tile([P, chunk], F32, tag="xt")
            ti = tpool.tile([P, chunk], I32, tag="ti")
            nc.sync.dma_start(out=xt, in_=xv[:, sl])
            nc.sync.dma_start(out=ti, in_=tv[:, sl])
            if stages < 1:
                return
            # u = x + t, column-split between GPSIMD (its one fast op,
            # plain tensor_tensor add) and DVE (fused (x-0)+t stt).
            u = upool.tile([P, chunk], F32, tag="u")
            gc = min(gp_cols, chunk)
            if gc > 0:
                nc.gpsimd.tensor_add(u[:, 0:gc], xt[:, 0:gc], ti[:, 0:gc])
            if gc < chunk:
                nc.vector.scalar_tensor_tensor(
                    out=u[:, gc:chunk], in0=xt[:, gc:chunk], scalar=0.0,
                    in1=ti[:, gc:chunk], op0=Alu.add, op1=Alu.add,
                )
            # r = |1 - u|  (walrus has no DVE abs; ACT Abs is the only one)
            r = upool.tile([P, chunk], F32, tag="r")
            nc.scalar.activation(r, u, Act.Abs, bias=1.0, scale=-1.0)
            if stages < 2:
                return
            l0 = bpool.tile([P, chunk], BF16, tag="l0")
            nc.scalar.activation(l0, r, Act.Ln)
            s = bpool.tile([P, chunk], BF16, tag="s")
            nc.scalar.activation(s, r, Act.Square, bias=1.0, scale=-1.0)
            if stages < 3:
                return
            g = bpool.tile([P, chunk], BF16, tag="g")
            nc.vector.tensor_scalar(
                out=g, in0=ti, scalar1=-1.0, scalar2=0.5,
                op0=Alu.mult, op1=Alu.add,
            )
            # E = max(l0, log 1e-5) * s  (fused clamp+mult)
            e = bpool.tile([P, chunk], BF16, tag="e")
            nc.vector.scalar_tensor_tensor(
                out=e, in0=l0, scalar=LOG_LO, in1=s,
                op0=Alu.max, op1=Alu.mult,
            )
            m2 = bpool.tile([P, chunk], BF16, tag="m2")
            nc.vector.tensor_mul(m2, e, g)
            if stages < 4:
                return
            for j in range(chunk // NMM):
                js = slice(j * NMM, (j + 1) * NMM)
                first = ci == 0 and j == 0
                last = ci == nch - 1 and j == chunk // NMM - 1
                nc.tensor.matmul(
                    ps_e[0:1, :], ones, e[:, js], start=first, stop=last
                )
                nc.tensor.matmul(
                    ps_m[0:1, :], ones, m2[:, js], start=first, stop=last
                )

        def chunk_v4(ci, nch, ps_e, ps_m):
            """Cast-DMA loads (bf16 lands directly), all-bf16 DVE chain,
            abs column-split between ACT (Abs) and DVE (sign-multiply)."""
            sl = slice(ci * chunk, (ci + 1) * chunk)
            xb = apool.tile([P, chunk], BF16, tag="xb")
            tb = tpool.tile([P, chunk], BF16, tag="tb")
            nc.gpsimd.dma_start(out=xb, in_=xv[:, sl])   # f32 -> bf16
            nc.gpsimd.dma_start(out=tb, in_=tv[:, sl])   # i32 -> bf16
            if stages < 1:
                return
            # sgn = 2t - 1 in {-1, +1}
            sg = upool.tile([P, chunk], BF16, tag="sg")
            nc.vector.tensor_scalar(
                out=sg, in0=tb, scalar1=2.0, scalar2=-1.0,
                op0=Alu.mult, op1=Alu.add,
            )
            # v = (x - 1) + t  (fused; |v| = r)
            v = upool.tile([P, chunk], BF16, tag="v")
            nc.vector.scalar_tensor_tensor(
                out=v, in0=xb, scalar=1.0, in1=tb,
                op0=Alu.subtract, op1=Alu.add,
            )
            # r = |v|: ACT Abs for the first gp_cols columns, DVE v*sgn
            # for the rest — balances the two engines.
            r = upool.tile([P, chunk], BF16, tag="r")
            ac = min(gp_cols, chunk)
            if ac > 0:
                nc.scalar.activation(r[:, 0:ac], v[:, 0:ac], Act.Abs)
            if ac < chunk:
                nc.vector.tensor_mul(
                    r[:, ac:chunk], v[:, ac:chunk], sg[:, ac:chunk]
                )
            if stages < 2:
                return
            l0 = bpool.tile([P, chunk], BF16, tag="l0")
            nc.scalar.activation(l0, r, Act.Ln)
            s = bpool.tile([P, chunk], BF16, tag="s")
            nc.scalar.activation(s, r, Act.Square, bias=1.0, scale=-1.0)
            if stages < 3:
                return
            e = bpool.tile([P, chunk], BF16, tag="e")
            nc.vector.scalar_tensor_tensor(
                out=e, in0=l0, scalar=LOG_LO, in1=s,
                op0=Alu.max, op1=Alu.mult,
            )
            m2 = bpool.tile([P, chunk], BF16, tag="m2")
            nc.vector.tensor_mul(m2, e, sg)
            if stages < 4:
                return
            for j in range(chunk // NMM):
                js = slice(j * NMM, (j + 1) * NMM)
                first = ci == 0 and j == 0
                last = ci == nch - 1 and j == chunk // NMM - 1
                nc.tensor.matmul(
                    ps_e[0:1, :], ones, e[:, js], start=first, stop=last
                )
                nc.tensor.matmul(
                    ps_m[0:1, :], ones, m2[:, js], start=first, stop=last
                )

        def _split(frac):
            """Columns [0:c] go to ACT, [c:chunk] to DVE; 64-aligned."""
            c = int(round(frac * chunk / 64.0)) * 64
            return max(0, min(chunk, c))

        DT16 = F16 if in_dt == "f16" else BF16

        big = {}

        def front_v6(ci):
            """SWDGE cast loads (f32->f16, i32->f16); w = t - x so that
            |w| = 1-r (Ln arg), w^2 = (1-r)^2 (no abs for the square), and
            the 1e-5 clamp folds into the Ln bias.  a/s column-split
            between ACT and DVE per aq/sq.  dma_chunk > chunk loads in
            bigger transfers that compute slices into."""
            if dma_chunk > chunk:
                gsz = dma_chunk // chunk
                g = ci // gsz
                if ci % gsz == 0:
                    gsl = slice(g * dma_chunk, (g + 1) * dma_chunk)
                    bx = apool.tile([P, dma_chunk], DT16, tag="bx")
                    bt = tpool.tile([P, dma_chunk], DT16, tag="bt")
                    nc.gpsimd.dma_start(out=bx, in_=xv[:, gsl])
                    nc.gpsimd.dma_start(out=bt, in_=tv[:, gsl])
                    big[g] = (bx, bt)
                bx, bt = big[g]
                off = (ci % gsz) * chunk
                xb = bx[:, off:off + chunk]
                tb = bt[:, off:off + chunk]
            else:
                sl = slice(ci * chunk, (ci + 1) * chunk)
                xb = apool.tile([P, chunk], DT16, tag="xb")
                tb = tpool.tile([P, chunk], DT16, tag="tb")
                nc.gpsimd.dma_start(out=xb, in_=xv[:, sl])
                nc.gpsimd.dma_start(out=tb, in_=tv[:, sl])
            if stages < 1:
                return None
            # w = t - x  (sign(w) = 2t-1; |w| = 1-r; w^2 = (1-r)^2)
            w = upool.tile([P, chunk], DT16, tag="w")
            nc.vector.scalar_tensor_tensor(
                out=w, in0=xb, scalar=-1.0, in1=tb, op0=Alu.mult, op1=Alu.add
            )
            if stages < 2:
                return None
            # a = |w|: ACT Abs on [0:ac], DVE max(-w, w) stt on [ac:]
            # (no sign tensor needed anywhere: m2 masks with tb directly)
            a = upool.tile([P, chunk], DT16, tag="a")
            ac = _split(aq)
            if ac > 0:
                nc.scalar.activation(a[:, 0:ac], w[:, 0:ac], Act.Abs)
            if ac < chunk:
                nc.vector.scalar_tensor_tensor(
                    out=a[:, ac:chunk], in0=w[:, ac:chunk], scalar=-1.0,
                    in1=w[:, ac:chunk], op0=Alu.mult, op1=Alu.max,
                )
            # s = w^2: ACT Square on [0:sc], DVE w*w on [sc:]
            s = bpool.tile([P, chunk], DT16, tag="s")
            sc = _split(sq)
            if sc > 0:
                nc.scalar.activation(s[:, 0:sc], w[:, 0:sc], Act.Square)
            if sc < chunk:
                nc.vector.tensor_mul(s[:, sc:chunk], w[:, sc:chunk],
                                     w[:, sc:chunk])
            # l0 = ln(1 + 1e-5 - |w|)   (clamp built into the bias)
            l0 = bpool.tile([P, chunk], DT16, tag="l0")
            nc.scalar.activation(l0, a, Act.Ln, bias=lnb, scale=-1.0)
            return (l0, s, tb)

        def front_v7(ci):
            """HWDGE raw loads (f32/i32, full DMA rate); one f32 stt does
            the merge + cast to f16; g2 = (w>=0) = t so the second
            accumulator is t*E directly (combine mode 'te')."""
            sl = slice(ci * chunk, (ci + 1) * chunk)
            xt = apool.tile([P, chunk], F32, tag="xt")
            ti = tpool.tile([P, chunk], I32, tag="ti")
            nc.sync.dma_start(out=xt, in_=xv[:, sl])
            nc.sync.dma_start(out=ti, in_=tv[:, sl])
            if stages < 1:
                return None
            # w = t - x in f16 (the only f32-rate DVE op)
            w = upool.tile([P, chunk], DT16, tag="w")
            nc.vector.scalar_tensor_tensor(
                out=w, in0=xt, scalar=-1.0, in1=ti, op0=Alu.mult, op1=Alu.add
            )
            # g2 = (w >= 0) = t in {0,1}  (4x ts off the f16 w)
            g2 = upool.tile([P, chunk], DT16, tag="g2")
            nc.vector.tensor_scalar(
                out=g2, in0=w, scalar1=0.0, scalar2=None,
                op0=Alu.is_ge, op1=Alu.bypass,
            )
            if stages < 2:
                return None
            # a = |w|: ACT Abs on [0:ac], DVE max(-w, w) stt on [ac:]
            a = upool.tile([P, chunk], DT16, tag="a")
            ac = _split(aq)
            if ac > 0:
                nc.scalar.activation(a[:, 0:ac], w[:, 0:ac], Act.Abs)
            if ac < chunk:
                nc.vector.scalar_tensor_tensor(
                    out=a[:, ac:chunk], in0=w[:, ac:chunk], scalar=-1.0,
                    in1=w[:, ac:chunk], op0=Alu.mult, op1=Alu.max,
                )
            s = bpool.tile([P, chunk], DT16, tag="s")
            sc = _split(sq)
            if sc > 0:
                nc.scalar.activation(s[:, 0:sc], w[:, 0:sc], Act.Square)
            if sc < chunk:
                nc.vector.tensor_mul(s[:, sc:chunk], w[:, sc:chunk],
                                     w[:, sc:chunk])
            l0 = bpool.tile([P, chunk], DT16, tag="l0")
            nc.scalar.activation(l0, a, Act.Ln, bias=lnb, scale=-1.0)
            return (l0, s, g2)

        def front_v8(ci):
            """u = x + t computed BY THE DMA: x loads raw f32 via HWDGE,
            then t DMAs into the SAME tile via SWDGE with i32->f32 cast +
            CCE accum-add.  No merge op on any engine.  r = |u-1| is the
            Ln argument directly; s = (1-r)^2; g2 = (u>=1) = t masks m2."""
            sl = slice(ci * chunk, (ci + 1) * chunk)
            u = apool.tile([P, chunk], F32, tag="u")
            nc.sync.dma_start(out=u, in_=xv[:, sl])
            nc.gpsimd.dma_start(out=u, in_=tv[:, sl], accum_op=Alu.add)
            if stages < 1:
                return None
            # g2 = (u >= 1) = t in {0,1}
            g2 = upool.tile([P, chunk], DT16, tag="g2")
            nc.vector.tensor_scalar(
                out=g2, in0=u, scalar1=1.0, scalar2=None,
                op0=Alu.is_ge, op1=Alu.bypass,
            )
            if stages < 2:
                return None
            # a = r = |u - 1|: ACT Abs(u-1) on [0:ac]; DVE d=u-1 then
            # max(-d, d) on [ac:]
            a = upool.tile([P, chunk], DT16, tag="a")
            ac = _split(aq)
            if ac > 0:
                nc.scalar.activation(a[:, 0:ac], u[:, 0:ac], Act.Abs,
                                     bias=m1b, scale=1.0)
            if ac < chunk:
                d = upool.tile([P, chunk], DT16, tag="d")
                nc.vector.tensor_scalar(
                    out=d[:, ac:chunk], in0=u[:, ac:chunk], scalar1=1.0,
                    scalar2=-1.0, op0=Alu.mult, op1=Alu.add,
                )
                nc.vector.scalar_tensor_tensor(
                    out=a[:, ac:chunk], in0=d[:, ac:chunk], scalar=-1.0,
                    in1=d[:, ac:chunk], op0=Alu.mult, op1=Alu.max,
                )
            # s = (1 - r)^2: ACT Square(1-a) on [0:sc]; DVE d2=1-a then
            # d2*d2 on [sc:]
            s = bpool.tile([P, chunk], DT16, tag="s")
            sc = _split(sq)
            if sc > 0:
                nc.scalar.activation(s[:, 0:sc], a[:, 0:sc], Act.Square,
                                     bias=1.0, scale=-1.0)
            if sc < chunk:
                d2 = bpool.tile([P, chunk], DT16, tag="d2")
                nc.vector.tensor_scalar(
                    out=d2[:, sc:chunk], in0=a[:, sc:chunk], scalar1=-1.0,
                    scalar2=1.0, op0=Alu.mult, op1=Alu.add,
                )
                nc.vector.tensor_mul(s[:, sc:chunk], d2[:, sc:chunk],
                                     d2[:, sc:chunk])
            # l0 = ln(r + 1e-5)  (clamp built into the bias)
            l0 = bpool.tile([P, chunk], DT16, tag="l0")
            nc.scalar.activation(l0, a, Act.Ln, bias=epsb, scale=1.0)
            return (l0, s, g2)

        def back_v67(ci, nch, ps_e, ps_m, st):
            if stages < 3 or st is None:
                return
            l0, s, sg = st
            e = bpool.tile([P, chunk], DT16, tag="e")
            nc.vector.tensor_mul(e, l0, s)
            m2 = bpool.tile([P, chunk], DT16, tag="m2")
            nc.vector.tensor_mul(m2, e, sg)
            if stages < 4:
                return
            for j in range(chunk // NMM):
                js = slice(j * NMM, (j + 1) * NMM)
                first = ci == 0 and j == 0
                last = ci == nch - 1 and j == chunk // NMM - 1
                nc.tensor.matmul(
                    ps_e[0:1, :], ones, e[:, js], start=first, stop=last
                )
                nc.tensor.matmul(
                    ps_m[0:1, :], ones, m2[:, js], start=first, stop=last
                )

        def chunk_eng(ci, nch, ps_e, ps_m):
            """Engine micro-bench: raw loads + n_act chained ACT passes +
            (one i32->f16 ts cast + n_dve chained f16 tt passes), the two
            chains independent.  Output is garbage; timing only."""
            sl = slice(ci * chunk, (ci + 1) * chunk)
            xt = apool.tile([P, chunk], F32, tag="xt")
            ti = tpool.tile([P, chunk], I32, tag="ti")
            nc.sync.dma_start(out=xt, in_=xv[:, sl])
            nc.sync.dma_start(out=ti, in_=tv[:, sl])
            prev = xt
            for k in range(n_act):
                nxt = bpool.tile([P, chunk], DT16, tag=f"ac{k}")
                if k == 0:
                    nc.scalar.activation(nxt, prev, Act.Abs)
                elif k == 1:
                    nc.scalar.activation(nxt, prev, Act.Ln,
                                         bias=lnb, scale=-1.0)
                else:
                    nc.scalar.activation(nxt, prev, Act.Square)
                prev = nxt
            dprev = upool.tile([P, chunk], DT16, tag="dc")
            nc.vector.tensor_scalar(
                out=dprev, in0=ti, scalar1=1.0, scalar2=None,
                op0=Alu.mult, op1=Alu.bypass,
            )
            for k in range(n_dve):
                dn = upool.tile([P, chunk], DT16, tag=f"dv{k}")
                nc.vector.tensor_mul(dn, dprev, dprev)
                dprev = dn

        chunk_fn = {
            "v2": chunk_v2, "gp_u": chunk_gp_u, "v3": chunk_v3,
            "v4": chunk_v4, "eng": chunk_eng,
        }.get(variant)
        front_fn = {"v6": front_v6, "v7": front_v7, "v8": front_v8}.get(variant)
        nch = FD // chunk
        assert FD % chunk == 0 and chunk % NMM == 0 and chunk % 20 == 0

        def body():
            for rep in range(reps):
                ps_e = ps_m = None
                if stages >= 4:
                    ps_e = psum.tile([1, NMM], F32, tag="ps_e")
                    ps_m = psum.tile([1, NMM], F32, tag="ps_m")
                if front_fn is not None:
                    # software-pipelined: back stage runs `skew` chunks
                    # behind the front stage so DVE/ACT always have
                    # independent work instead of alternating in lockstep.
                    st = {}
                    for ci in range(nch + skew):
                        if ci < nch:
                            st[ci] = front_fn(ci)
                        if ci >= skew:
                            back_v67(ci - skew, nch, ps_e, ps_m,
                                     st.pop(ci - skew))
                else:
                    for ci in range(nch):
                        chunk_fn(ci, nch, ps_e, ps_m)
                res = opool.tile([1, 2 * NMM], F32, tag="res")
                if stages >= 4:
                    if res_on == "act":
                        # ScE is closer to PSUM (172 vs 120+ cyc fixed) and
                        # less loaded than DVE here; Copy is in every table
                        # set and takes a float bias.
                        nc.scalar.activation(
                            res[0:1, 0:NMM], ps_e[0:1, :], Act.Copy)
                        nc.scalar.activation(
                            res[0:1, NMM : 2 * NMM], ps_m[0:1, :], Act.Copy)
                    else:
                        nc.vector.tensor_copy(res[0:1, 0:NMM], ps_e[0:1, :])
                        nc.vector.tensor_copy(
                            res[0:1, NMM : 2 * NMM], ps_m[0:1, :])
                else:
                    nc.vector.memset(res, 0.0)
                nc.sync.dma_start(out=out[:], in_=res)

        if loop_n > 0:
            with tc.For_i(0, loop_n, 1, staggered_reset=staggered):
                body()
        else:
            body()

    nc.finalize()
    return nc


_NC_CACHE: dict = {}


def _get_nc(**kw) -> bacc.Bacc:
    key = tuple(sorted(kw.items()))
    if key not in _NC_CACHE:
        _NC_CACHE[key] = build_bass(**kw)
    return _NC_CACHE[key]


def combine_partials(partials, m2_scale: float = M2_SCALE["v2"],
                     m2_mode: str = "me") -> np.float32:
    """Host-side reduction of the per-core [1, 2*NMM] partial sums.

    m2_mode "me": second accumulator holds +-colsum((1-2t)E) (scaled);
    m2_mode "te": second accumulator holds colsum(t*E) directly (v7).
    """
    cs_e = np.zeros(C, dtype=np.float64)
    cs_m2 = np.zeros(C, dtype=np.float64)
    cols = np.arange(NMM) % C
    for p in partials:
        p = np.asarray(p, dtype=np.float64).reshape(2 * NMM)
        np.add.at(cs_e, cols, p[:NMM])
        np.add.at(cs_m2, cols, p[NMM:])
    if m2_mode == "te":
        cs_te = m2_scale * cs_m2        # colsum(t * E)
        cs_me = cs_e - 2.0 * cs_te      # colsum((1-2t) * E)
    else:
        cs_me = m2_scale * cs_m2        # colsum((1-2t) * E)
        cs_te = (cs_e - cs_me) / 2.0    # colsum(t * E)
    total = (-SS * cs_te - W * cs_me).sum()
    return np.float32(10.0 * total / (SS * B * C))


def kernel(output: np.ndarray, target: np.ndarray) -> np.ndarray:
    output = np.ascontiguousarray(np.asarray(output, dtype=np.float32))
    target = np.ascontiguousarray(np.asarray(target, dtype=np.int32))
    assert output.shape == (B, C) and target.shape == (B, C)

    nc = _get_nc()
    xs = output.reshape(NCORES, BS, C)
    ts = target.reshape(NCORES, BS, C)
    in_maps = [{"output": xs[i], "target": ts[i]} for i in range(NCORES)]
    res = run_bass_kernel_spmd(nc, in_maps, core_ids=list(range(NCORES)))
    return np.asarray(
        combine_partials(
            [res.results[i]["partials"] for i in range(NCORES)],
            m2_scale=M2_SCALE[DEFAULT_VARIANT],
            m2_mode="te" if DEFAULT_VARIANT in ("v6", "v7", "v8") else "me",
        )
    )

